# revision 14
# baseline (speedup 1.0000x reference)
"""Trainium2 Bass kernel for the GNN ExplainModule (masked adjacency).

Strategy (8 NeuronCores, row-sharded output):
  - Each core owns 1250 rows of the [10000, 10000] output, processed in
    row-blocks of 128. Host routes each edge's two contributions
    ((r,c) and (c,r), weight 0.5*gate) to the owning core/block.
  - Host pre-gathers per-token operands (index routing / data layout
    only — all FP math runs on device):
      xab[:, t] = [embed[row_t]; embed[col_t]]  (stacked, transposed)
      av[t] = adj[r_t, c_t], nz[t] = noise, cm[t] = c_t % MW
  - Contributions within a block are merged into MW-wide destination
    segments: one scatter token per occupied (row, col//MW) segment, so
    segments are unique per scatter instruction (no CCE races, no
    waves). Contributions are ranked within their segment; the MLP
    token stream is rank-major with each rank padded to 128 so rank r
    of segment-slot s sits at stream position off_r*128 + s (slots
    sorted by segment population, so each rank occupies a dense slot
    prefix).
  - Device MLP (weight-stationary): preT[64, n] = W1ab_folded^T @ xab
    (fp32r, 512-wide moving tiles), relu+c_vec-bias on Scalar engine,
    PE-transpose back to token-partition layout [128 tok, 64].
    W1ab/b1/c host-permuted (W2>=0 first) and scaled by |W2| so the W2
    stage is reduce(pos) - reduce(neg). gate = sigmoid(logit(nz)+s+b2).
  - payload[128, Sg, MW]: rank 0 initializes via onehot(cm)*gm, ranks
    >=1 accumulate over their slot-prefix; one dma_scatter_add per
    block into the pre-zeroed output (CCE add); pads target a per-block
    pad segment with zero payload.
"""

import sys

import numpy as np

for _p in ("/opt/trn_rl_repo",):
    if _p not in sys.path:
        sys.path.insert(0, _p)

N = 10000
D = 64
NCORES = 8
RPC = N // NCORES  # rows per core
BLK = 128  # rows per block
MW = 128  # merge width (scatter elem size, f32; bytes must be %256)
NSEGW = -(-N // MW)  # real MW-wide segments per row
SEGW = NSEGW + 1  # +1 pad segment (all-zero scatter target)
PITCH = SEGW * MW  # row pitch in the output slab


def _blocks():
    out = []
    r = 0
    while r < RPC:
        h = min(BLK, RPC - r)
        out.append((r, h))
        r += h
    return out


def _prep_host(row, col, noise, embed, adj):
    """Route contributions to (core, block, segment-group, rank)."""
    row = np.asarray(row).astype(np.int64).ravel()
    col = np.asarray(col).astype(np.int64).ravel()
    noise = np.asarray(noise).astype(np.float32).ravel()

    dr = np.concatenate([row, col])  # dest row
    dc = np.concatenate([col, row])  # dest col
    ea = np.concatenate([row, row])  # MLP first input index (edge row)
    eb = np.concatenate([col, col])  # MLP second input index (edge col)
    en = np.concatenate([noise, noise])
    av_all = np.asarray(adj)[dr, dc].astype(np.float32)
    core = dr // RPC

    blocks = _blocks()
    nblk = len(blocks)
    pad_si = NSEGW  # row 0's pad segment; never holds real data

    # Pass 1: per (core, block) group contributions into MW-segments,
    # rank within segment, slot = position of segment in count-desc order.
    info = [[None] * nblk for _ in range(NCORES)]
    for k in range(NCORES):
        m = core == k
        rl = dr[m] - k * RPC
        dcc, a, b, nz, av = dc[m], ea[m], eb[m], en[m], av_all[m]
        blk_id = rl // BLK
        for bi, (r0, h) in enumerate(blocks):
            sel = blk_id == bi
            rls = rl[sel] - r0
            dcs = dcc[sel]
            gsi = rls * SEGW + dcs // MW
            o = np.argsort(gsi, kind="stable")
            gsi_s = gsi[o]
            uq, inv, cnt = np.unique(
                gsi_s, return_inverse=True, return_counts=True
            )
            starts = np.zeros(len(uq) + 1, np.int64)
            np.cumsum(cnt, out=starts[1:])
            rank = np.arange(len(gsi_s)) - starts[inv]
            gord = np.argsort(-cnt, kind="stable")  # groups by count desc
            slot_of_group = np.empty(len(uq), np.int64)
            slot_of_group[gord] = np.arange(len(uq))
            slot = slot_of_group[inv]
            cnt_sorted = cnt[gord]
            maxrank = int(cnt_sorted[0]) if len(cnt_sorted) else 0
            n_j = [int((cnt_sorted > j).sum()) for j in range(maxrank)]
            info[k][bi] = dict(
                a=a[sel][o], b=b[sel][o], nz=nz[sel][o],
                cm=(dcs[o] % MW).astype(np.float32),
                av=av[sel][o], rank=rank, slot=slot, n_j=n_j,
                si_tok=uq[gord], G=len(uq),
            )

    # Pass 2: SPMD-static sizes per block
    chunks = []
    o1 = o16 = o128 = 0
    for bi, (r0, h) in enumerate(blocks):
        Tg = max(info[k][bi]["G"] for k in range(NCORES))
        Tg = max(-(-Tg // 128) * 128, 128)
        Sg0 = Tg // 128
        maxrank = max(len(info[k][bi]["n_j"]) for k in range(NCORES))
        rank_cols = []
        off = Sg0
        for j in range(1, maxrank):
            nj = max(
                (info[k][bi]["n_j"][j] if j < len(info[k][bi]["n_j"]) else 0)
                for k in range(NCORES)
            )
            ncols = -(-nj // 128)
            if ncols <= 0:
                continue
            rank_cols.append((j, off, ncols))
            off += ncols
        S = off
        t = S * 128
        chunks.append(dict(
            bi=bi, r0b=bi * BLK, S=S, Sg0=Sg0, Tg=Tg,
            rank_cols=rank_cols, t=t, o1=o1, o16=o16, o128=o128,
        ))
        o1 += t
        o16 += Tg // 16
        o128 += S
    total1, total16, total128 = o1, o16, o128

    embed = np.asarray(embed, dtype=np.float32)
    embT = np.ascontiguousarray(embed.T)  # [D, N]

    per_core = []
    for k in range(NCORES):
        xab = np.zeros((2 * D, total1), np.float32)
        si16 = np.full((128, total16), pad_si, np.int16)
        nzf = np.full((128, total128), 0.5, np.float32)
        cmf = np.zeros((128, total128), np.float32)
        avf = np.zeros((128, total128), np.float32)
        for ch in chunks:
            nfo = info[k][ch["bi"]]
            t, o1, o16, o128 = ch["t"], ch["o1"], ch["o16"], ch["o128"]
            # stream: rank-major, slot position within rank
            a = np.zeros(t, np.int64)
            b = np.zeros(t, np.int64)
            nz = np.full(t, 0.5, np.float32)
            cm = np.zeros(t, np.float32)
            av = np.zeros(t, np.float32)
            col_off = {0: 0}
            for j, off, ncols in ch["rank_cols"]:
                col_off[j] = off
            for j in range(len(nfo["n_j"])):
                if j not in col_off:
                    continue
                sel = nfo["rank"] == j
                pos = col_off[j] * 128 + nfo["slot"][sel]
                a[pos] = nfo["a"][sel]
                b[pos] = nfo["b"][sel]
                nz[pos] = nfo["nz"][sel]
                cm[pos] = nfo["cm"][sel]
                av[pos] = nfo["av"][sel]
            xab[:D, o1 : o1 + t] = embT[:, a]
            xab[D:, o1 : o1 + t] = embT[:, b]
            si = np.full(ch["Tg"], pad_si, np.int64)
            si[: nfo["G"]] = nfo["si_tok"]
            si16[:, o16 : o16 + ch["Tg"] // 16] = np.tile(
                np.ascontiguousarray(si.reshape(-1, 16).T), (8, 1)
            ).astype(np.int16)
            S = ch["S"]
            nzf[:, o128 : o128 + S] = np.ascontiguousarray(nz.reshape(-1, 128).T)
            cmf[:, o128 : o128 + S] = np.ascontiguousarray(cm.reshape(-1, 128).T)
            avf[:, o128 : o128 + S] = np.ascontiguousarray(av.reshape(-1, 128).T)
        per_core.append(dict(xab=xab, si16=si16, nz=nzf, cm=cmf, av=avf))
    return per_core, chunks, total1, total16, total128


def _build_program(chunks, total1, total16, total128, b2f, pos_cnt):
    import concourse.bacc as bacc
    import concourse.mybir as mybir
    import concourse.tile as tile
    from concourse.masks import make_identity

    f32 = mybir.dt.float32
    bf16 = mybir.dt.bfloat16
    f32r = mybir.dt.float32r
    i16 = mybir.dt.int16
    add = mybir.AluOpType.add
    mult = mybir.AluOpType.mult
    subtract = mybir.AluOpType.subtract
    is_equal = mybir.AluOpType.is_equal
    AF = mybir.ActivationFunctionType

    nc = bacc.Bacc(num_swdge_queues=4)

    blocks = _blocks()
    out_rows = BLK * len(blocks)

    xabp = nc.declare_dram_parameter("xab", [2 * D, total1], f32r, isOutput=False)
    sip = nc.declare_dram_parameter("si16", [128, total16], i16, isOutput=False)
    nzp = nc.declare_dram_parameter("nz", [128, total128], f32, isOutput=False)
    cmp_ = nc.declare_dram_parameter("cm", [128, total128], f32, isOutput=False)
    avp = nc.declare_dram_parameter("av", [128, total128], f32, isOutput=False)
    w1p = nc.declare_dram_parameter("w1abf", [2 * D, D], f32r, isOutput=False)
    w1cp = nc.declare_dram_parameter("w1cf", [D, D], f32, isOutput=False)
    b1p = nc.declare_dram_parameter("b1f", [1, D], f32, isOutput=False)
    e5p = nc.declare_dram_parameter("e5t", [D, 1], f32, isOutput=False)
    iop = nc.declare_dram_parameter("iotaw", [128, MW], f32, isOutput=False)
    outps = [
        nc.declare_dram_parameter(f"out{bi}", [BLK, PITCH], f32, isOutput=True)
        for bi in range(len(blocks))
    ]

    MMT = 512  # moving-dim tile for the W1 matmul

    with tile.TileContext(nc) as tc:
        with (
            tc.tile_pool(name="const", bufs=1) as cp,
            tc.tile_pool(name="xin", bufs=2) as xp,
            tc.tile_pool(name="hts", bufs=2) as hp,
            tc.tile_pool(name="work", bufs=2) as wp,
            tc.tile_pool(name="sidx", bufs=4) as sp,
            tc.tile_pool(name="smal", bufs=3) as mp,
            tc.tile_pool(name="pay", bufs=2) as yp,
            tc.tile_pool(name="tmp", bufs=1) as tp,
            tc.tile_pool(name="psa", bufs=2, space="PSUM") as ppa,
            tc.tile_pool(name="psb", bufs=4, space="PSUM") as ppb,
            tc.tile_pool(name="psc", bufs=1, space="PSUM") as ppc,
        ):
            identity = cp.tile([128, 128], f32)
            make_identity(nc, identity[:])
            w1ab = cp.tile([2 * D, D], f32r)
            nc.sync.dma_start(out=w1ab[:], in_=w1p[:, :])
            w1c = cp.tile([D, D], f32)
            nc.sync.dma_start(out=w1c[:], in_=w1cp[:, :])
            b1t = cp.tile([1, D], f32)
            nc.sync.dma_start(out=b1t[:], in_=b1p[:, :])
            e5 = cp.tile([D, 1], f32)
            nc.sync.dma_start(out=e5[:], in_=e5p[:, :])
            iot = cp.tile([128, MW], f32)
            nc.sync.dma_start(out=iot[:], in_=iop[:, :])

            # c_vec = embed[node_idx] @ W1c_folded + b1_folded -> [64, 1]
            cps = ppc.tile([1, D], f32, tag="cps")
            nc.tensor.matmul(cps[:], lhsT=e5[:], rhs=w1c[:], start=True, stop=True)
            crow = cp.tile([1, D], f32)
            nc.vector.tensor_tensor(out=crow[:], in0=cps[:], in1=b1t[:], op=add)
            cpsT = ppc.tile([D, 1], f32, tag="cpsT")
            nc.tensor.transpose(cpsT[:], crow[:], identity[:1, :1])
            cT = cp.tile([D, 1], f32)
            nc.scalar.copy(out=cT[:], in_=cpsT[:])

            for ch in chunks:
                S, Sg0, Tg, t = ch["S"], ch["Sg0"], ch["Tg"], ch["t"]
                o1, o16, o128, r0b = ch["o1"], ch["o16"], ch["o128"], ch["r0b"]

                nz = mp.tile([128, S], f32, tag="nz")
                nc.sync.dma_start(out=nz[:], in_=nzp[:, o128 : o128 + S])
                cm = mp.tile([128, S], f32, tag="cm")
                nc.sync.dma_start(out=cm[:], in_=cmp_[:, o128 : o128 + S])
                av = mp.tile([128, S], f32, tag="av")
                nc.sync.dma_start(out=av[:], in_=avp[:, o128 : o128 + S])

                # MLP in two sub-chunks to bound SBUF
                h = wp.tile([128, S * D], f32, tag="h")
                sii = sp.tile([128, Tg // 16], i16, tag="sii")
                Sa = -(-S // 2)
                for (c0, cS) in ((0, Sa), (Sa, S - Sa)):
                    if cS <= 0:
                        continue
                    ta = cS * 128
                    xt = xp.tile([2 * D, ta], f32r, tag="xt")
                    nc.sync.dma_start(
                        out=xt[:], in_=xabp[:, o1 + c0 * 128 : o1 + c0 * 128 + ta]
                    )
                    hT = hp.tile([D, ta], f32, tag="hT")
                    for j0 in range(0, ta, MMT):
                        n = min(MMT, ta - j0)
                        psA = ppa.tile([D, MMT], f32, tag="psA")
                        nc.tensor.matmul(
                            psA[:, :n],
                            lhsT=w1ab[:],
                            rhs=xt[:, j0 : j0 + n],
                            start=True,
                            stop=True,
                        )
                        nc.scalar.activation(
                            out=hT[:, j0 : j0 + n], in_=psA[:, :n],
                            func=AF.Relu, bias=cT[:],
                        )
                    for g0 in range(0, cS, 4):
                        gn = min(4, cS - g0)
                        psB = ppb.tile([128, 4 * D], f32, tag="psB")
                        for q in range(gn):
                            nc.tensor.transpose(
                                psB[:, q * D : (q + 1) * D],
                                hT[:, (g0 + q) * 128 : (g0 + q + 1) * 128],
                                identity[:D, :D],
                            )
                        nc.scalar.copy(
                            out=h[:, (c0 + g0) * D : (c0 + g0 + gn) * D],
                            in_=psB[:, : gn * D],
                        )

                nc.sync.dma_start(out=sii[:], in_=sip[:, o16 : o16 + Tg // 16])

                h3 = h[:].rearrange("p (s d) -> p s d", d=D)
                s = wp.tile([128, S], f32, tag="s")
                if pos_cnt == D:
                    nc.vector.tensor_reduce(
                        out=s[:], in_=h3, axis=mybir.AxisListType.X, op=add
                    )
                elif pos_cnt == 0:
                    nc.vector.tensor_reduce(
                        out=s[:], in_=h3, axis=mybir.AxisListType.X, op=add,
                        negate=True,
                    )
                else:
                    nc.vector.tensor_reduce(
                        out=s[:], in_=h3[:, :, :pos_cnt],
                        axis=mybir.AxisListType.X, op=add,
                    )
                    sn = wp.tile([128, S], f32, tag="sn")
                    nc.vector.tensor_reduce(
                        out=sn[:], in_=h3[:, :, pos_cnt:],
                        axis=mybir.AxisListType.X, op=add,
                    )
                    nc.vector.tensor_tensor(
                        out=s[:], in0=s[:], in1=sn[:], op=subtract
                    )

                # gate = sigmoid(ln(nz) - ln(1-nz) + s + b2); gm = 0.5*g*av
                om = wp.tile([128, S], f32, tag="om")
                nc.vector.tensor_scalar(
                    out=om[:], in0=nz[:], scalar1=-1.0, scalar2=1.0,
                    op0=mult, op1=add,
                )
                ln1 = wp.tile([128, S], f32, tag="ln1")
                nc.scalar.activation(out=ln1[:], in_=nz[:], func=AF.Ln)
                ln2 = wp.tile([128, S], f32, tag="ln2")
                nc.scalar.activation(out=ln2[:], in_=om[:], func=AF.Ln)
                z = wp.tile([128, S], f32, tag="z")
                nc.vector.scalar_tensor_tensor(
                    out=z[:], in0=ln1[:], scalar=b2f, in1=ln2[:],
                    op0=add, op1=subtract,
                )
                nc.vector.tensor_tensor(out=z[:], in0=z[:], in1=s[:], op=add)
                g_ = wp.tile([128, S], f32, tag="g")
                nc.scalar.activation(out=g_[:], in_=z[:], func=AF.Sigmoid)
                gm = wp.tile([128, S], f32, tag="gm")
                nc.vector.scalar_tensor_tensor(
                    out=gm[:], in0=g_[:], scalar=0.5, in1=av[:],
                    op0=mult, op1=mult,
                )

                # payload: rank0 initializes, ranks >=1 accumulate prefix
                pay = yp.tile([128, Sg0 * MW], f32, tag="pay")
                pay3 = pay[:].rearrange("p (s w) -> p s w", w=MW)
                io_b = iot[:].rearrange("p (o w) -> p o w", o=1)
                nc.vector.tensor_tensor(
                    out=pay3,
                    in0=io_b.to_broadcast([128, Sg0, MW]),
                    in1=cm[:, :Sg0].rearrange("p (s o) -> p s o", o=1)
                    .to_broadcast([128, Sg0, MW]),
                    op=is_equal,
                )
                nc.vector.tensor_tensor(
                    out=pay3,
                    in0=pay3,
                    in1=gm[:, :Sg0].rearrange("p (s o) -> p s o", o=1)
                    .to_broadcast([128, Sg0, MW]),
                    op=mult,
                )
                maxnc = max((nc_ for _, _, nc_ in ch["rank_cols"]), default=0)
                for j, off, ncols in ch["rank_cols"]:
                    tmp = tp.tile([128, maxnc * MW], f32, tag="tmp")
                    tmp3 = tmp[:, : ncols * MW].rearrange(
                        "p (s w) -> p s w", w=MW
                    )
                    nc.vector.tensor_tensor(
                        out=tmp3,
                        in0=io_b.to_broadcast([128, ncols, MW]),
                        in1=cm[:, off : off + ncols]
                        .rearrange("p (s o) -> p s o", o=1)
                        .to_broadcast([128, ncols, MW]),
                        op=is_equal,
                    )
                    nc.vector.tensor_tensor(
                        out=tmp3,
                        in0=tmp3,
                        in1=gm[:, off : off + ncols]
                        .rearrange("p (s o) -> p s o", o=1)
                        .to_broadcast([128, ncols, MW]),
                        op=mult,
                    )
                    nc.vector.tensor_tensor(
                        out=pay3[:, :ncols, :],
                        in0=pay3[:, :ncols, :],
                        in1=tmp3,
                        op=add,
                    )

                out_view = outps[ch["bi"]][:, :].rearrange(
                    "p (s w) -> (p s) w", w=MW
                )
                nc.gpsimd.dma_scatter_add(
                    out_ap=out_view,
                    in_ap=pay3,
                    idxs_ap=sii[:],
                    num_idxs=Tg,
                    num_idxs_reg=Tg,
                    elem_size=MW,
                    queue_num=ch["bi"] % 4,
                )

    nc.compile()
    return nc


def kernel(embed, row, col, adj, noise, W1, b1, W2, b2, node_idx):
    from concourse.bass_utils import run_bass_kernel_spmd

    embed = np.ascontiguousarray(np.asarray(embed), dtype=np.float32)
    adj = np.ascontiguousarray(np.asarray(adj), dtype=np.float32)
    W1 = np.ascontiguousarray(np.asarray(W1), dtype=np.float32)
    b1 = np.ascontiguousarray(np.asarray(b1), dtype=np.float32).ravel()
    W2 = np.ascontiguousarray(np.asarray(W2), dtype=np.float32)
    b2f = float(np.asarray(b2, dtype=np.float32).ravel()[0])
    nidx = int(np.asarray(node_idx))

    # permute hidden units (W2 >= 0 first) and fold |W2| into W1/b1 so
    # the W2 stage becomes reduce(pos) - reduce(neg) after relu
    w2v = W2.reshape(-1).astype(np.float32)
    order = np.argsort(w2v < 0, kind="stable")
    pos_cnt = int((w2v >= 0).sum())
    w2a = np.abs(w2v[order]).reshape(1, D)
    W1f = W1[:, order] * w2a  # [3D, D]
    b1f = (b1[order].reshape(1, D) * w2a).astype(np.float32)
    w1abf = np.ascontiguousarray(W1f[: 2 * D])
    w1cf = np.ascontiguousarray(W1f[2 * D :])
    e5t = np.ascontiguousarray(embed[nidx].reshape(D, 1))
    iotaw = np.ascontiguousarray(
        np.tile(np.arange(MW, dtype=np.float32).reshape(1, MW), (128, 1))
    )

    per_core, chunks, total1, total16, total128 = _prep_host(
        row, col, noise, embed, adj
    )
    nc = _build_program(chunks, total1, total16, total128, b2f, pos_cnt)

    blocks = _blocks()
    in_maps = []
    for k in range(NCORES):
        m = dict(per_core[k])
        m.update(w1abf=w1abf, w1cf=w1cf, b1f=b1f, e5t=e5t, iotaw=iotaw)
        in_maps.append(m)

    res = run_bass_kernel_spmd(nc, in_maps, list(range(NCORES)))
    kernel.last_exec_time_ns = res.exec_time_ns
    pieces = []
    for k in range(NCORES):
        for bi, (r0, h) in enumerate(blocks):
            pieces.append(res.results[k][f"out{bi}"][:h, :N])
    out = np.concatenate(pieces, axis=0)
    return out


kernel.last_exec_time_ns = None


# revision 15
# speedup vs baseline: 1.1569x; 1.1569x over previous
"""Trainium2 Bass kernel for the GNN ExplainModule (masked adjacency).

Strategy (8 NeuronCores, row-sharded output):
  - Each core owns 1250 rows of the [10000, 10000] output, processed in
    row-blocks of 128. Host routes each edge's two contributions
    ((r,c) and (c,r), weight 0.5*gate) to the owning core/block.
  - Host pre-gathers per-token operands (index routing / data layout
    only — all FP math runs on device):
      xab[:, t] = [embed[row_t]; embed[col_t]]  (stacked, transposed)
      av[t] = adj[r_t, c_t], nz[t] = noise, cm[t] = c_t % MW
  - Contributions within a block are merged into MW-wide destination
    segments: one scatter token per occupied (row, col//MW) segment, so
    segments are unique per scatter instruction (no CCE races, no
    waves). Contributions are ranked within their segment; the MLP
    token stream is rank-major with each rank padded to 128 so rank r
    of segment-slot s sits at stream position off_r*128 + s (slots
    sorted by segment population, so each rank occupies a dense slot
    prefix).
  - Device MLP (weight-stationary): preT[64, n] = W1ab_folded^T @ xab
    (fp32r, 512-wide moving tiles), relu+c_vec-bias on Scalar engine,
    PE-transpose back to token-partition layout [128 tok, 64].
    W1ab/b1/c host-permuted (W2>=0 first) and scaled by |W2| so the W2
    stage is reduce(pos) - reduce(neg). gate = sigmoid(logit(nz)+s+b2).
  - payload[128, Sg, MW]: rank 0 initializes via onehot(cm)*gm, ranks
    >=1 accumulate over their slot-prefix; one dma_scatter_add per
    block into the pre-zeroed output (CCE add); pads target a per-block
    pad segment with zero payload.
"""

import sys

import numpy as np

for _p in ("/opt/trn_rl_repo",):
    if _p not in sys.path:
        sys.path.insert(0, _p)

N = 10000
D = 64
NCORES = 8
RPC = N // NCORES  # rows per core
BLK = 64  # rows per block
MW = 128  # merge width (scatter elem size, f32; bytes must be %256)
NSEGW = -(-N // MW)  # real MW-wide segments per row
SEGW = NSEGW + 1  # +1 pad segment (all-zero scatter target)
PITCH = SEGW * MW  # row pitch in the output slab


def _blocks():
    out = []
    r = 0
    while r < RPC:
        h = min(BLK, RPC - r)
        out.append((r, h))
        r += h
    return out


def _prep_host(row, col, noise, embed, adj):
    """Route contributions to (core, block, segment-group, rank)."""
    row = np.asarray(row).astype(np.int64).ravel()
    col = np.asarray(col).astype(np.int64).ravel()
    noise = np.asarray(noise).astype(np.float32).ravel()

    dr = np.concatenate([row, col])  # dest row
    dc = np.concatenate([col, row])  # dest col
    ea = np.concatenate([row, row])  # MLP first input index (edge row)
    eb = np.concatenate([col, col])  # MLP second input index (edge col)
    en = np.concatenate([noise, noise])
    av_all = np.asarray(adj)[dr, dc].astype(np.float32)
    core = dr // RPC

    blocks = _blocks()
    nblk = len(blocks)
    pad_si = NSEGW  # row 0's pad segment; never holds real data

    # Pass 1: per (core, block) group contributions into MW-segments,
    # rank within segment, slot = position of segment in count-desc order.
    info = [[None] * nblk for _ in range(NCORES)]
    for k in range(NCORES):
        m = core == k
        rl = dr[m] - k * RPC
        dcc, a, b, nz, av = dc[m], ea[m], eb[m], en[m], av_all[m]
        blk_id = rl // BLK
        for bi, (r0, h) in enumerate(blocks):
            sel = blk_id == bi
            rls = rl[sel] - r0
            dcs = dcc[sel]
            gsi = rls * SEGW + dcs // MW
            o = np.argsort(gsi, kind="stable")
            gsi_s = gsi[o]
            uq, inv, cnt = np.unique(
                gsi_s, return_inverse=True, return_counts=True
            )
            starts = np.zeros(len(uq) + 1, np.int64)
            np.cumsum(cnt, out=starts[1:])
            rank = np.arange(len(gsi_s)) - starts[inv]
            gord = np.argsort(-cnt, kind="stable")  # groups by count desc
            slot_of_group = np.empty(len(uq), np.int64)
            slot_of_group[gord] = np.arange(len(uq))
            slot = slot_of_group[inv]
            cnt_sorted = cnt[gord]
            maxrank = int(cnt_sorted[0]) if len(cnt_sorted) else 0
            n_j = [int((cnt_sorted > j).sum()) for j in range(maxrank)]
            info[k][bi] = dict(
                a=a[sel][o], b=b[sel][o], nz=nz[sel][o],
                cm=(dcs[o] % MW).astype(np.float32),
                av=av[sel][o], rank=rank, slot=slot, n_j=n_j,
                si_tok=uq[gord], G=len(uq),
            )

    # Pass 2: SPMD-static sizes per block
    chunks = []
    o1 = o16 = o128 = 0
    for bi, (r0, h) in enumerate(blocks):
        Tg = max(info[k][bi]["G"] for k in range(NCORES))
        Tg = max(-(-Tg // 128) * 128, 128)
        Sg0 = Tg // 128
        maxrank = max(len(info[k][bi]["n_j"]) for k in range(NCORES))
        rank_cols = []
        off = Sg0
        for j in range(1, maxrank):
            nj = max(
                (info[k][bi]["n_j"][j] if j < len(info[k][bi]["n_j"]) else 0)
                for k in range(NCORES)
            )
            ncols = -(-nj // 128)
            if ncols <= 0:
                continue
            rank_cols.append((j, off, ncols))
            off += ncols
        S = off
        t = S * 128
        chunks.append(dict(
            bi=bi, r0b=bi * BLK, S=S, Sg0=Sg0, Tg=Tg,
            rank_cols=rank_cols, t=t, o1=o1, o16=o16, o128=o128,
        ))
        o1 += t
        o16 += Tg // 16
        o128 += S
    total1, total16, total128 = o1, o16, o128

    embed = np.asarray(embed, dtype=np.float32)
    embT = np.ascontiguousarray(embed.T)  # [D, N]

    per_core = []
    for k in range(NCORES):
        xab = np.zeros((2 * D, total1), np.float32)
        si16 = np.full((128, total16), pad_si, np.int16)
        nzf = np.full((128, total128), 0.5, np.float32)
        cmf = np.zeros((128, total128), np.float32)
        avf = np.zeros((128, total128), np.float32)
        for ch in chunks:
            nfo = info[k][ch["bi"]]
            t, o1, o16, o128 = ch["t"], ch["o1"], ch["o16"], ch["o128"]
            # stream: rank-major, slot position within rank
            a = np.zeros(t, np.int64)
            b = np.zeros(t, np.int64)
            nz = np.full(t, 0.5, np.float32)
            cm = np.zeros(t, np.float32)
            av = np.zeros(t, np.float32)
            col_off = {0: 0}
            for j, off, ncols in ch["rank_cols"]:
                col_off[j] = off
            for j in range(len(nfo["n_j"])):
                if j not in col_off:
                    continue
                sel = nfo["rank"] == j
                pos = col_off[j] * 128 + nfo["slot"][sel]
                a[pos] = nfo["a"][sel]
                b[pos] = nfo["b"][sel]
                nz[pos] = nfo["nz"][sel]
                cm[pos] = nfo["cm"][sel]
                av[pos] = nfo["av"][sel]
            xab[:D, o1 : o1 + t] = embT[:, a]
            xab[D:, o1 : o1 + t] = embT[:, b]
            si = np.full(ch["Tg"], pad_si, np.int64)
            si[: nfo["G"]] = nfo["si_tok"]
            si16[:, o16 : o16 + ch["Tg"] // 16] = np.tile(
                np.ascontiguousarray(si.reshape(-1, 16).T), (8, 1)
            ).astype(np.int16)
            S = ch["S"]
            nzf[:, o128 : o128 + S] = np.ascontiguousarray(nz.reshape(-1, 128).T)
            cmf[:, o128 : o128 + S] = np.ascontiguousarray(cm.reshape(-1, 128).T)
            avf[:, o128 : o128 + S] = np.ascontiguousarray(av.reshape(-1, 128).T)
        per_core.append(dict(xab=xab, si16=si16, nz=nzf, cm=cmf, av=avf))
    return per_core, chunks, total1, total16, total128


def _build_program(chunks, total1, total16, total128, b2f, pos_cnt):
    import concourse.bacc as bacc
    import concourse.mybir as mybir
    import concourse.tile as tile
    from concourse.masks import make_identity

    f32 = mybir.dt.float32
    bf16 = mybir.dt.bfloat16
    f32r = mybir.dt.float32r
    i16 = mybir.dt.int16
    add = mybir.AluOpType.add
    mult = mybir.AluOpType.mult
    subtract = mybir.AluOpType.subtract
    is_equal = mybir.AluOpType.is_equal
    AF = mybir.ActivationFunctionType

    nc = bacc.Bacc(num_swdge_queues=4)

    blocks = _blocks()
    out_rows = BLK * len(blocks)

    xabp = nc.declare_dram_parameter("xab", [2 * D, total1], f32r, isOutput=False)
    sip = nc.declare_dram_parameter("si16", [128, total16], i16, isOutput=False)
    nzp = nc.declare_dram_parameter("nz", [128, total128], f32, isOutput=False)
    cmp_ = nc.declare_dram_parameter("cm", [128, total128], f32, isOutput=False)
    avp = nc.declare_dram_parameter("av", [128, total128], f32, isOutput=False)
    w1p = nc.declare_dram_parameter("w1abf", [2 * D, D], f32r, isOutput=False)
    w1cp = nc.declare_dram_parameter("w1cf", [D, D], f32, isOutput=False)
    b1p = nc.declare_dram_parameter("b1f", [1, D], f32, isOutput=False)
    e5p = nc.declare_dram_parameter("e5t", [D, 1], f32, isOutput=False)
    iop = nc.declare_dram_parameter("iotaw", [128, MW], f32, isOutput=False)
    outps = [
        nc.declare_dram_parameter(f"out{bi}", [BLK, PITCH], f32, isOutput=True)
        for bi in range(len(blocks))
    ]

    MMT = 512  # moving-dim tile for the W1 matmul

    with tile.TileContext(nc) as tc:
        with (
            tc.tile_pool(name="const", bufs=1) as cp,
            tc.tile_pool(name="xin", bufs=3) as xp,
            tc.tile_pool(name="hts", bufs=3) as hp,
            tc.tile_pool(name="work", bufs=3) as wp,
            tc.tile_pool(name="sidx", bufs=4) as sp,
            tc.tile_pool(name="smal", bufs=3) as mp,
            tc.tile_pool(name="pay", bufs=3) as yp,
            tc.tile_pool(name="tmp", bufs=2) as tp,
            tc.tile_pool(name="psa", bufs=2, space="PSUM") as ppa,
            tc.tile_pool(name="psb", bufs=4, space="PSUM") as ppb,
            tc.tile_pool(name="psc", bufs=1, space="PSUM") as ppc,
        ):
            identity = cp.tile([128, 128], f32)
            make_identity(nc, identity[:])
            w1ab = cp.tile([2 * D, D], f32r)
            nc.sync.dma_start(out=w1ab[:], in_=w1p[:, :])
            w1c = cp.tile([D, D], f32)
            nc.sync.dma_start(out=w1c[:], in_=w1cp[:, :])
            b1t = cp.tile([1, D], f32)
            nc.sync.dma_start(out=b1t[:], in_=b1p[:, :])
            e5 = cp.tile([D, 1], f32)
            nc.sync.dma_start(out=e5[:], in_=e5p[:, :])
            iot = cp.tile([128, MW], f32)
            nc.sync.dma_start(out=iot[:], in_=iop[:, :])

            # c_vec = embed[node_idx] @ W1c_folded + b1_folded -> [64, 1]
            cps = ppc.tile([1, D], f32, tag="cps")
            nc.tensor.matmul(cps[:], lhsT=e5[:], rhs=w1c[:], start=True, stop=True)
            crow = cp.tile([1, D], f32)
            nc.vector.tensor_tensor(out=crow[:], in0=cps[:], in1=b1t[:], op=add)
            cpsT = ppc.tile([D, 1], f32, tag="cpsT")
            nc.tensor.transpose(cpsT[:], crow[:], identity[:1, :1])
            cT = cp.tile([D, 1], f32)
            nc.scalar.copy(out=cT[:], in_=cpsT[:])

            for ch in chunks:
                S, Sg0, Tg, t = ch["S"], ch["Sg0"], ch["Tg"], ch["t"]
                o1, o16, o128, r0b = ch["o1"], ch["o16"], ch["o128"], ch["r0b"]

                nz = mp.tile([128, S], f32, tag="nz")
                nc.sync.dma_start(out=nz[:], in_=nzp[:, o128 : o128 + S])
                cm = mp.tile([128, S], f32, tag="cm")
                nc.sync.dma_start(out=cm[:], in_=cmp_[:, o128 : o128 + S])
                av = mp.tile([128, S], f32, tag="av")
                nc.sync.dma_start(out=av[:], in_=avp[:, o128 : o128 + S])

                # MLP in two sub-chunks to bound SBUF
                h = wp.tile([128, S * D], f32, tag="h")
                sii = sp.tile([128, Tg // 16], i16, tag="sii")
                Sa = -(-S // 2)
                for (c0, cS) in ((0, Sa), (Sa, S - Sa)):
                    if cS <= 0:
                        continue
                    ta = cS * 128
                    xt = xp.tile([2 * D, ta], f32r, tag="xt")
                    nc.sync.dma_start(
                        out=xt[:], in_=xabp[:, o1 + c0 * 128 : o1 + c0 * 128 + ta]
                    )
                    hT = hp.tile([D, ta], f32, tag="hT")
                    for j0 in range(0, ta, MMT):
                        n = min(MMT, ta - j0)
                        psA = ppa.tile([D, MMT], f32, tag="psA")
                        nc.tensor.matmul(
                            psA[:, :n],
                            lhsT=w1ab[:],
                            rhs=xt[:, j0 : j0 + n],
                            start=True,
                            stop=True,
                        )
                        nc.scalar.activation(
                            out=hT[:, j0 : j0 + n], in_=psA[:, :n],
                            func=AF.Relu, bias=cT[:],
                        )
                    for g0 in range(0, cS, 4):
                        gn = min(4, cS - g0)
                        psB = ppb.tile([128, 4 * D], f32, tag="psB")
                        for q in range(gn):
                            nc.tensor.transpose(
                                psB[:, q * D : (q + 1) * D],
                                hT[:, (g0 + q) * 128 : (g0 + q + 1) * 128],
                                identity[:D, :D],
                            )
                        nc.scalar.copy(
                            out=h[:, (c0 + g0) * D : (c0 + g0 + gn) * D],
                            in_=psB[:, : gn * D],
                        )

                nc.sync.dma_start(out=sii[:], in_=sip[:, o16 : o16 + Tg // 16])

                h3 = h[:].rearrange("p (s d) -> p s d", d=D)
                s = wp.tile([128, S], f32, tag="s")
                if pos_cnt == D:
                    nc.vector.tensor_reduce(
                        out=s[:], in_=h3, axis=mybir.AxisListType.X, op=add
                    )
                elif pos_cnt == 0:
                    nc.vector.tensor_reduce(
                        out=s[:], in_=h3, axis=mybir.AxisListType.X, op=add,
                        negate=True,
                    )
                else:
                    nc.vector.tensor_reduce(
                        out=s[:], in_=h3[:, :, :pos_cnt],
                        axis=mybir.AxisListType.X, op=add,
                    )
                    sn = wp.tile([128, S], f32, tag="sn")
                    nc.vector.tensor_reduce(
                        out=sn[:], in_=h3[:, :, pos_cnt:],
                        axis=mybir.AxisListType.X, op=add,
                    )
                    nc.vector.tensor_tensor(
                        out=s[:], in0=s[:], in1=sn[:], op=subtract
                    )

                # gate = sigmoid(ln(nz) - ln(1-nz) + s + b2); gm = 0.5*g*av
                om = wp.tile([128, S], f32, tag="om")
                nc.vector.tensor_scalar(
                    out=om[:], in0=nz[:], scalar1=-1.0, scalar2=1.0,
                    op0=mult, op1=add,
                )
                ln1 = wp.tile([128, S], f32, tag="ln1")
                nc.scalar.activation(out=ln1[:], in_=nz[:], func=AF.Ln)
                ln2 = wp.tile([128, S], f32, tag="ln2")
                nc.scalar.activation(out=ln2[:], in_=om[:], func=AF.Ln)
                z = wp.tile([128, S], f32, tag="z")
                nc.vector.scalar_tensor_tensor(
                    out=z[:], in0=ln1[:], scalar=b2f, in1=ln2[:],
                    op0=add, op1=subtract,
                )
                nc.vector.tensor_tensor(out=z[:], in0=z[:], in1=s[:], op=add)
                g_ = wp.tile([128, S], f32, tag="g")
                nc.scalar.activation(out=g_[:], in_=z[:], func=AF.Sigmoid)
                gm = wp.tile([128, S], f32, tag="gm")
                nc.vector.scalar_tensor_tensor(
                    out=gm[:], in0=g_[:], scalar=0.5, in1=av[:],
                    op0=mult, op1=mult,
                )

                # payload: rank0 initializes, ranks >=1 accumulate prefix
                pay = yp.tile([128, Sg0 * MW], f32, tag="pay")
                pay3 = pay[:].rearrange("p (s w) -> p s w", w=MW)
                io_b = iot[:].rearrange("p (o w) -> p o w", o=1)
                nc.vector.tensor_tensor(
                    out=pay3,
                    in0=io_b.to_broadcast([128, Sg0, MW]),
                    in1=cm[:, :Sg0].rearrange("p (s o) -> p s o", o=1)
                    .to_broadcast([128, Sg0, MW]),
                    op=is_equal,
                )
                nc.vector.tensor_tensor(
                    out=pay3,
                    in0=pay3,
                    in1=gm[:, :Sg0].rearrange("p (s o) -> p s o", o=1)
                    .to_broadcast([128, Sg0, MW]),
                    op=mult,
                )
                maxnc = max((nc_ for _, _, nc_ in ch["rank_cols"]), default=0)
                for j, off, ncols in ch["rank_cols"]:
                    tmp = tp.tile([128, maxnc * MW], f32, tag="tmp")
                    tmp3 = tmp[:, : ncols * MW].rearrange(
                        "p (s w) -> p s w", w=MW
                    )
                    nc.vector.tensor_tensor(
                        out=tmp3,
                        in0=io_b.to_broadcast([128, ncols, MW]),
                        in1=cm[:, off : off + ncols]
                        .rearrange("p (s o) -> p s o", o=1)
                        .to_broadcast([128, ncols, MW]),
                        op=is_equal,
                    )
                    nc.vector.tensor_tensor(
                        out=tmp3,
                        in0=tmp3,
                        in1=gm[:, off : off + ncols]
                        .rearrange("p (s o) -> p s o", o=1)
                        .to_broadcast([128, ncols, MW]),
                        op=mult,
                    )
                    nc.vector.tensor_tensor(
                        out=pay3[:, :ncols, :],
                        in0=pay3[:, :ncols, :],
                        in1=tmp3,
                        op=add,
                    )

                out_view = outps[ch["bi"]][:, :].rearrange(
                    "p (s w) -> (p s) w", w=MW
                )
                nc.gpsimd.dma_scatter_add(
                    out_ap=out_view,
                    in_ap=pay3,
                    idxs_ap=sii[:],
                    num_idxs=Tg,
                    num_idxs_reg=Tg,
                    elem_size=MW,
                    queue_num=ch["bi"] % 4,
                )

    nc.compile()
    return nc


def kernel(embed, row, col, adj, noise, W1, b1, W2, b2, node_idx):
    from concourse.bass_utils import run_bass_kernel_spmd

    embed = np.ascontiguousarray(np.asarray(embed), dtype=np.float32)
    adj = np.ascontiguousarray(np.asarray(adj), dtype=np.float32)
    W1 = np.ascontiguousarray(np.asarray(W1), dtype=np.float32)
    b1 = np.ascontiguousarray(np.asarray(b1), dtype=np.float32).ravel()
    W2 = np.ascontiguousarray(np.asarray(W2), dtype=np.float32)
    b2f = float(np.asarray(b2, dtype=np.float32).ravel()[0])
    nidx = int(np.asarray(node_idx))

    # permute hidden units (W2 >= 0 first) and fold |W2| into W1/b1 so
    # the W2 stage becomes reduce(pos) - reduce(neg) after relu
    w2v = W2.reshape(-1).astype(np.float32)
    order = np.argsort(w2v < 0, kind="stable")
    pos_cnt = int((w2v >= 0).sum())
    w2a = np.abs(w2v[order]).reshape(1, D)
    W1f = W1[:, order] * w2a  # [3D, D]
    b1f = (b1[order].reshape(1, D) * w2a).astype(np.float32)
    w1abf = np.ascontiguousarray(W1f[: 2 * D])
    w1cf = np.ascontiguousarray(W1f[2 * D :])
    e5t = np.ascontiguousarray(embed[nidx].reshape(D, 1))
    iotaw = np.ascontiguousarray(
        np.tile(np.arange(MW, dtype=np.float32).reshape(1, MW), (128, 1))
    )

    per_core, chunks, total1, total16, total128 = _prep_host(
        row, col, noise, embed, adj
    )
    nc = _build_program(chunks, total1, total16, total128, b2f, pos_cnt)

    blocks = _blocks()
    in_maps = []
    for k in range(NCORES):
        m = dict(per_core[k])
        m.update(w1abf=w1abf, w1cf=w1cf, b1f=b1f, e5t=e5t, iotaw=iotaw)
        in_maps.append(m)

    res = run_bass_kernel_spmd(nc, in_maps, list(range(NCORES)))
    kernel.last_exec_time_ns = res.exec_time_ns
    pieces = []
    for k in range(NCORES):
        for bi, (r0, h) in enumerate(blocks):
            pieces.append(res.results[k][f"out{bi}"][:h, :N])
    out = np.concatenate(pieces, axis=0)
    return out


kernel.last_exec_time_ns = None


# revision 16
# speedup vs baseline: 1.2601x; 1.0892x over previous
"""Trainium2 Bass kernel for the GNN ExplainModule (masked adjacency).

Strategy (8 NeuronCores, row-sharded output):
  - Each core owns 1250 rows of the [10000, 10000] output, processed in
    row-blocks of 128. Host routes each edge's two contributions
    ((r,c) and (c,r), weight 0.5*gate) to the owning core/block.
  - Host pre-gathers per-token operands (index routing / data layout
    only — all FP math runs on device):
      xab[:, t] = [embed[row_t]; embed[col_t]]  (stacked, transposed)
      av[t] = adj[r_t, c_t], nz[t] = noise, cm[t] = c_t % MW
  - Contributions within a block are merged into MW-wide destination
    segments: one scatter token per occupied (row, col//MW) segment, so
    segments are unique per scatter instruction (no CCE races, no
    waves). Contributions are ranked within their segment; the MLP
    token stream is rank-major with each rank padded to 128 so rank r
    of segment-slot s sits at stream position off_r*128 + s (slots
    sorted by segment population, so each rank occupies a dense slot
    prefix).
  - Device MLP (weight-stationary): preT[64, n] = W1ab_folded^T @ xab
    (fp32r, 512-wide moving tiles), relu+c_vec-bias on Scalar engine,
    PE-transpose back to token-partition layout [128 tok, 64].
    W1ab/b1/c host-permuted (W2>=0 first) and scaled by |W2| so the W2
    stage is reduce(pos) - reduce(neg). gate = sigmoid(logit(nz)+s+b2).
  - payload[128, Sg, MW]: rank 0 initializes via onehot(cm)*gm, ranks
    >=1 accumulate over their slot-prefix; one dma_scatter_add per
    block into the pre-zeroed output (CCE add); pads target a per-block
    pad segment with zero payload.
"""

import sys

import numpy as np

for _p in ("/opt/trn_rl_repo",):
    if _p not in sys.path:
        sys.path.insert(0, _p)

N = 10000
D = 64
NCORES = 8
RPC = N // NCORES  # rows per core
BLK = 64  # rows per block
MW = 128  # merge width (scatter elem size, f32; bytes must be %256)
NSEGW = -(-N // MW)  # real MW-wide segments per row
SEGW = NSEGW + 1  # +1 pad segment (all-zero scatter target)
PITCH = SEGW * MW  # row pitch in the output slab


def _blocks():
    out = []
    r = 0
    while r < RPC:
        h = min(BLK, RPC - r)
        out.append((r, h))
        r += h
    return out


def _prep_host(row, col, noise, embed, adj):
    """Route contributions to (core, block, segment-group, rank)."""
    row = np.asarray(row).astype(np.int64).ravel()
    col = np.asarray(col).astype(np.int64).ravel()
    noise = np.asarray(noise).astype(np.float32).ravel()

    dr = np.concatenate([row, col])  # dest row
    dc = np.concatenate([col, row])  # dest col
    ea = np.concatenate([row, row])  # MLP first input index (edge row)
    eb = np.concatenate([col, col])  # MLP second input index (edge col)
    en = np.concatenate([noise, noise])
    av_all = np.asarray(adj)[dr, dc].astype(np.float32)
    core = dr // RPC

    blocks = _blocks()
    nblk = len(blocks)
    pad_si = NSEGW  # row 0's pad segment; never holds real data

    # Pass 1: per (core, block) group contributions into MW-segments,
    # rank within segment, slot = position of segment in count-desc order.
    info = [[None] * nblk for _ in range(NCORES)]
    for k in range(NCORES):
        m = core == k
        rl = dr[m] - k * RPC
        dcc, a, b, nz, av = dc[m], ea[m], eb[m], en[m], av_all[m]
        blk_id = rl // BLK
        for bi, (r0, h) in enumerate(blocks):
            sel = blk_id == bi
            rls = rl[sel] - r0
            dcs = dcc[sel]
            gsi = rls * SEGW + dcs // MW
            o = np.argsort(gsi, kind="stable")
            gsi_s = gsi[o]
            uq, inv, cnt = np.unique(
                gsi_s, return_inverse=True, return_counts=True
            )
            starts = np.zeros(len(uq) + 1, np.int64)
            np.cumsum(cnt, out=starts[1:])
            rank = np.arange(len(gsi_s)) - starts[inv]
            gord = np.argsort(-cnt, kind="stable")  # groups by count desc
            slot_of_group = np.empty(len(uq), np.int64)
            slot_of_group[gord] = np.arange(len(uq))
            slot = slot_of_group[inv]
            cnt_sorted = cnt[gord]
            maxrank = int(cnt_sorted[0]) if len(cnt_sorted) else 0
            n_j = [int((cnt_sorted > j).sum()) for j in range(maxrank)]
            info[k][bi] = dict(
                a=a[sel][o], b=b[sel][o], nz=nz[sel][o],
                cm=(dcs[o] % MW).astype(np.float32),
                av=av[sel][o], rank=rank, slot=slot, n_j=n_j,
                si_tok=uq[gord], G=len(uq),
            )

    # Pass 2: SPMD-static sizes per block
    chunks = []
    o1 = o16 = o128 = 0
    for bi, (r0, h) in enumerate(blocks):
        Tg = max(info[k][bi]["G"] for k in range(NCORES))
        Tg = max(-(-Tg // 128) * 128, 128)
        Sg0 = Tg // 128
        maxrank = max(len(info[k][bi]["n_j"]) for k in range(NCORES))
        rank_cols = []
        off = Sg0
        for j in range(1, maxrank):
            nj = max(
                (info[k][bi]["n_j"][j] if j < len(info[k][bi]["n_j"]) else 0)
                for k in range(NCORES)
            )
            ncols = -(-nj // 128)
            if ncols <= 0:
                continue
            rank_cols.append((j, off, ncols))
            off += ncols
        S = off
        t = S * 128
        chunks.append(dict(
            bi=bi, r0b=bi * BLK, S=S, Sg0=Sg0, Tg=Tg,
            rank_cols=rank_cols, t=t, o1=o1, o16=o16, o128=o128,
        ))
        o1 += t
        o16 += Tg // 16
        o128 += S
    total1, total16, total128 = o1, o16, o128

    embed = np.asarray(embed, dtype=np.float32)
    embT = np.ascontiguousarray(embed.T)  # [D, N]

    per_core = []
    for k in range(NCORES):
        xab = np.zeros((2 * D, total1), np.float32)
        si16 = np.full((128, total16), pad_si, np.int16)
        nzf = np.full((128, total128), 0.5, np.float32)
        cmf = np.zeros((128, total128), np.float32)
        avf = np.zeros((128, total128), np.float32)
        for ch in chunks:
            nfo = info[k][ch["bi"]]
            t, o1, o16, o128 = ch["t"], ch["o1"], ch["o16"], ch["o128"]
            # stream: rank-major, slot position within rank
            a = np.zeros(t, np.int64)
            b = np.zeros(t, np.int64)
            nz = np.full(t, 0.5, np.float32)
            cm = np.zeros(t, np.float32)
            av = np.zeros(t, np.float32)
            col_off = {0: 0}
            for j, off, ncols in ch["rank_cols"]:
                col_off[j] = off
            for j in range(len(nfo["n_j"])):
                if j not in col_off:
                    continue
                sel = nfo["rank"] == j
                pos = col_off[j] * 128 + nfo["slot"][sel]
                a[pos] = nfo["a"][sel]
                b[pos] = nfo["b"][sel]
                nz[pos] = nfo["nz"][sel]
                cm[pos] = nfo["cm"][sel]
                av[pos] = nfo["av"][sel]
            xab[:D, o1 : o1 + t] = embT[:, a]
            xab[D:, o1 : o1 + t] = embT[:, b]
            si = np.full(ch["Tg"], pad_si, np.int64)
            si[: nfo["G"]] = nfo["si_tok"]
            si16[:, o16 : o16 + ch["Tg"] // 16] = np.tile(
                np.ascontiguousarray(si.reshape(-1, 16).T), (8, 1)
            ).astype(np.int16)
            S = ch["S"]
            nzf[:, o128 : o128 + S] = np.ascontiguousarray(nz.reshape(-1, 128).T)
            cmf[:, o128 : o128 + S] = np.ascontiguousarray(cm.reshape(-1, 128).T)
            avf[:, o128 : o128 + S] = np.ascontiguousarray(av.reshape(-1, 128).T)
        per_core.append(dict(xab=xab, si16=si16, nz=nzf, cm=cmf, av=avf))
    return per_core, chunks, total1, total16, total128


def _build_program(chunks, total1, total16, total128, b2f, pos_cnt):
    import concourse.bacc as bacc
    import concourse.mybir as mybir
    import concourse.tile as tile
    from concourse.masks import make_identity

    f32 = mybir.dt.float32
    bf16 = mybir.dt.bfloat16
    f32r = mybir.dt.float32r
    i16 = mybir.dt.int16
    add = mybir.AluOpType.add
    mult = mybir.AluOpType.mult
    subtract = mybir.AluOpType.subtract
    is_equal = mybir.AluOpType.is_equal
    AF = mybir.ActivationFunctionType

    nc = bacc.Bacc(num_swdge_queues=4)

    blocks = _blocks()
    out_rows = BLK * len(blocks)

    xabp = nc.declare_dram_parameter("xab", [2 * D, total1], f32r, isOutput=False)
    sip = nc.declare_dram_parameter("si16", [128, total16], i16, isOutput=False)
    nzp = nc.declare_dram_parameter("nz", [128, total128], f32, isOutput=False)
    cmp_ = nc.declare_dram_parameter("cm", [128, total128], f32, isOutput=False)
    avp = nc.declare_dram_parameter("av", [128, total128], f32, isOutput=False)
    w1p = nc.declare_dram_parameter("w1abf", [2 * D, D], f32r, isOutput=False)
    w1cp = nc.declare_dram_parameter("w1cf", [D, D], f32, isOutput=False)
    b1p = nc.declare_dram_parameter("b1f", [1, D], f32, isOutput=False)
    e5p = nc.declare_dram_parameter("e5t", [D, 1], f32, isOutput=False)
    iop = nc.declare_dram_parameter("iotaw", [128, MW], f32, isOutput=False)
    outps = [
        nc.declare_dram_parameter(f"out{bi}", [BLK, PITCH], f32, isOutput=True)
        for bi in range(len(blocks))
    ]

    MMT = 512  # moving-dim tile for the W1 matmul

    with tile.TileContext(nc) as tc:
        with (
            tc.tile_pool(name="const", bufs=1) as cp,
            tc.tile_pool(name="xin", bufs=3) as xp,
            tc.tile_pool(name="hts", bufs=3) as hp,
            tc.tile_pool(name="work", bufs=3) as wp,
            tc.tile_pool(name="sidx", bufs=4) as sp,
            tc.tile_pool(name="smal", bufs=3) as mp,
            tc.tile_pool(name="pay", bufs=3) as yp,
            tc.tile_pool(name="psa", bufs=2, space="PSUM") as ppa,
            tc.tile_pool(name="psb", bufs=4, space="PSUM") as ppb,
            tc.tile_pool(name="psc", bufs=1, space="PSUM") as ppc,
        ):
            identity = cp.tile([128, 128], f32)
            make_identity(nc, identity[:])
            w1ab = cp.tile([2 * D, D], f32r)
            nc.sync.dma_start(out=w1ab[:], in_=w1p[:, :])
            w1c = cp.tile([D, D], f32)
            nc.sync.dma_start(out=w1c[:], in_=w1cp[:, :])
            b1t = cp.tile([1, D], f32)
            nc.sync.dma_start(out=b1t[:], in_=b1p[:, :])
            e5 = cp.tile([D, 1], f32)
            nc.sync.dma_start(out=e5[:], in_=e5p[:, :])
            iot = cp.tile([128, MW], f32)
            nc.sync.dma_start(out=iot[:], in_=iop[:, :])

            # c_vec = embed[node_idx] @ W1c_folded + b1_folded -> [64, 1]
            cps = ppc.tile([1, D], f32, tag="cps")
            nc.tensor.matmul(cps[:], lhsT=e5[:], rhs=w1c[:], start=True, stop=True)
            crow = cp.tile([1, D], f32)
            nc.vector.tensor_tensor(out=crow[:], in0=cps[:], in1=b1t[:], op=add)
            cpsT = ppc.tile([D, 1], f32, tag="cpsT")
            nc.tensor.transpose(cpsT[:], crow[:], identity[:1, :1])
            cT = cp.tile([D, 1], f32)
            nc.scalar.copy(out=cT[:], in_=cpsT[:])

            for ch in chunks:
                S, Sg0, Tg, t = ch["S"], ch["Sg0"], ch["Tg"], ch["t"]
                o1, o16, o128, r0b = ch["o1"], ch["o16"], ch["o128"], ch["r0b"]

                nz = mp.tile([128, S], f32, tag="nz")
                nc.sync.dma_start(out=nz[:], in_=nzp[:, o128 : o128 + S])
                cm = mp.tile([128, S], f32, tag="cm")
                nc.sync.dma_start(out=cm[:], in_=cmp_[:, o128 : o128 + S])
                av = mp.tile([128, S], f32, tag="av")
                nc.sync.dma_start(out=av[:], in_=avp[:, o128 : o128 + S])

                # MLP in two sub-chunks to bound SBUF
                h = wp.tile([128, S * D], f32, tag="h")
                sii = sp.tile([128, Tg // 16], i16, tag="sii")
                Sa = -(-S // 2)
                for (c0, cS) in ((0, Sa), (Sa, S - Sa)):
                    if cS <= 0:
                        continue
                    ta = cS * 128
                    xt = xp.tile([2 * D, ta], f32r, tag="xt")
                    nc.sync.dma_start(
                        out=xt[:], in_=xabp[:, o1 + c0 * 128 : o1 + c0 * 128 + ta]
                    )
                    hT = hp.tile([D, ta], f32, tag="hT")
                    for j0 in range(0, ta, MMT):
                        n = min(MMT, ta - j0)
                        psA = ppa.tile([D, MMT], f32, tag="psA")
                        nc.tensor.matmul(
                            psA[:, :n],
                            lhsT=w1ab[:],
                            rhs=xt[:, j0 : j0 + n],
                            start=True,
                            stop=True,
                        )
                        nc.scalar.activation(
                            out=hT[:, j0 : j0 + n], in_=psA[:, :n],
                            func=AF.Relu, bias=cT[:],
                        )
                    for g0 in range(0, cS, 4):
                        gn = min(4, cS - g0)
                        psB = ppb.tile([128, 4 * D], f32, tag="psB")
                        for q in range(gn):
                            nc.tensor.transpose(
                                psB[:, q * D : (q + 1) * D],
                                hT[:, (g0 + q) * 128 : (g0 + q + 1) * 128],
                                identity[:D, :D],
                            )
                        nc.scalar.copy(
                            out=h[:, (c0 + g0) * D : (c0 + g0 + gn) * D],
                            in_=psB[:, : gn * D],
                        )

                nc.sync.dma_start(out=sii[:], in_=sip[:, o16 : o16 + Tg // 16])

                h3 = h[:].rearrange("p (s d) -> p s d", d=D)
                s = wp.tile([128, S], f32, tag="s")
                if pos_cnt == D:
                    nc.vector.tensor_reduce(
                        out=s[:], in_=h3, axis=mybir.AxisListType.X, op=add
                    )
                elif pos_cnt == 0:
                    nc.vector.tensor_reduce(
                        out=s[:], in_=h3, axis=mybir.AxisListType.X, op=add,
                        negate=True,
                    )
                else:
                    nc.vector.tensor_reduce(
                        out=s[:], in_=h3[:, :, :pos_cnt],
                        axis=mybir.AxisListType.X, op=add,
                    )
                    sn = wp.tile([128, S], f32, tag="sn")
                    nc.vector.tensor_reduce(
                        out=sn[:], in_=h3[:, :, pos_cnt:],
                        axis=mybir.AxisListType.X, op=add,
                    )
                    nc.vector.tensor_tensor(
                        out=s[:], in0=s[:], in1=sn[:], op=subtract
                    )

                # gate = sigmoid(ln(nz) - ln(1-nz) + s + b2); gm = 0.5*g*av
                om = wp.tile([128, S], f32, tag="om")
                nc.vector.tensor_scalar(
                    out=om[:], in0=nz[:], scalar1=-1.0, scalar2=1.0,
                    op0=mult, op1=add,
                )
                ln1 = wp.tile([128, S], f32, tag="ln1")
                nc.scalar.activation(out=ln1[:], in_=nz[:], func=AF.Ln)
                ln2 = wp.tile([128, S], f32, tag="ln2")
                nc.scalar.activation(out=ln2[:], in_=om[:], func=AF.Ln)
                z = wp.tile([128, S], f32, tag="z")
                nc.vector.scalar_tensor_tensor(
                    out=z[:], in0=ln1[:], scalar=b2f, in1=ln2[:],
                    op0=add, op1=subtract,
                )
                nc.vector.tensor_tensor(out=z[:], in0=z[:], in1=s[:], op=add)
                g_ = wp.tile([128, S], f32, tag="g")
                nc.scalar.activation(out=g_[:], in_=z[:], func=AF.Sigmoid)
                gm = wp.tile([128, S], f32, tag="gm")
                nc.vector.scalar_tensor_tensor(
                    out=gm[:], in0=g_[:], scalar=0.5, in1=av[:],
                    op0=mult, op1=mult,
                )

                # payload: one full-width onehot*gm over all rank cols,
                # then in-place prefix adds fold rank regions into rank 0
                pay = yp.tile([128, S * MW], f32, tag="pay")
                payall = pay[:].rearrange("p (s w) -> p s w", w=MW)
                pay3 = payall[:, :Sg0, :]
                io_b = iot[:].rearrange("p (o w) -> p o w", o=1)
                nc.vector.tensor_tensor(
                    out=payall,
                    in0=io_b.to_broadcast([128, S, MW]),
                    in1=cm[:].rearrange("p (s o) -> p s o", o=1)
                    .to_broadcast([128, S, MW]),
                    op=is_equal,
                )
                nc.vector.tensor_tensor(
                    out=payall,
                    in0=payall,
                    in1=gm[:].rearrange("p (s o) -> p s o", o=1)
                    .to_broadcast([128, S, MW]),
                    op=mult,
                )
                for j, off, ncols in ch["rank_cols"]:
                    nc.vector.tensor_tensor(
                        out=payall[:, :ncols, :],
                        in0=payall[:, :ncols, :],
                        in1=payall[:, off : off + ncols, :],
                        op=add,
                    )

                out_view = outps[ch["bi"]][:, :].rearrange(
                    "p (s w) -> (p s) w", w=MW
                )
                nc.gpsimd.dma_scatter_add(
                    out_ap=out_view,
                    in_ap=pay3,
                    idxs_ap=sii[:],
                    num_idxs=Tg,
                    num_idxs_reg=Tg,
                    elem_size=MW,
                    queue_num=ch["bi"] % 4,
                )

    nc.compile()
    return nc


def kernel(embed, row, col, adj, noise, W1, b1, W2, b2, node_idx):
    from concourse.bass_utils import run_bass_kernel_spmd

    embed = np.ascontiguousarray(np.asarray(embed), dtype=np.float32)
    adj = np.ascontiguousarray(np.asarray(adj), dtype=np.float32)
    W1 = np.ascontiguousarray(np.asarray(W1), dtype=np.float32)
    b1 = np.ascontiguousarray(np.asarray(b1), dtype=np.float32).ravel()
    W2 = np.ascontiguousarray(np.asarray(W2), dtype=np.float32)
    b2f = float(np.asarray(b2, dtype=np.float32).ravel()[0])
    nidx = int(np.asarray(node_idx))

    # permute hidden units (W2 >= 0 first) and fold |W2| into W1/b1 so
    # the W2 stage becomes reduce(pos) - reduce(neg) after relu
    w2v = W2.reshape(-1).astype(np.float32)
    order = np.argsort(w2v < 0, kind="stable")
    pos_cnt = int((w2v >= 0).sum())
    w2a = np.abs(w2v[order]).reshape(1, D)
    W1f = W1[:, order] * w2a  # [3D, D]
    b1f = (b1[order].reshape(1, D) * w2a).astype(np.float32)
    w1abf = np.ascontiguousarray(W1f[: 2 * D])
    w1cf = np.ascontiguousarray(W1f[2 * D :])
    e5t = np.ascontiguousarray(embed[nidx].reshape(D, 1))
    iotaw = np.ascontiguousarray(
        np.tile(np.arange(MW, dtype=np.float32).reshape(1, MW), (128, 1))
    )

    per_core, chunks, total1, total16, total128 = _prep_host(
        row, col, noise, embed, adj
    )
    nc = _build_program(chunks, total1, total16, total128, b2f, pos_cnt)

    blocks = _blocks()
    in_maps = []
    for k in range(NCORES):
        m = dict(per_core[k])
        m.update(w1abf=w1abf, w1cf=w1cf, b1f=b1f, e5t=e5t, iotaw=iotaw)
        in_maps.append(m)

    res = run_bass_kernel_spmd(nc, in_maps, list(range(NCORES)))
    kernel.last_exec_time_ns = res.exec_time_ns
    pieces = []
    for k in range(NCORES):
        for bi, (r0, h) in enumerate(blocks):
            pieces.append(res.results[k][f"out{bi}"][:h, :N])
    out = np.concatenate(pieces, axis=0)
    return out


kernel.last_exec_time_ns = None


# revision 18
# speedup vs baseline: 1.5750x; 1.2499x over previous
"""Trainium2 Bass kernel for the GNN ExplainModule (masked adjacency).

Strategy (8 NeuronCores, row-sharded output):
  - Each core owns 1250 rows of the [10000, 10000] output, processed in
    row-blocks of 128. Host routes each edge's two contributions
    ((r,c) and (c,r), weight 0.5*gate) to the owning core/block.
  - Host pre-gathers per-token operands (index routing / data layout
    only — all FP math runs on device):
      xab[:, t] = [embed[row_t]; embed[col_t]]  (stacked, transposed)
      av[t] = adj[r_t, c_t], nz[t] = noise, cm[t] = c_t % MW
  - Contributions within a block are merged into MW-wide destination
    segments: one scatter token per occupied (row, col//MW) segment, so
    segments are unique per scatter instruction (no CCE races, no
    waves). Contributions are ranked within their segment; the MLP
    token stream is rank-major with each rank padded to 128 so rank r
    of segment-slot s sits at stream position off_r*128 + s (slots
    sorted by segment population, so each rank occupies a dense slot
    prefix).
  - Device MLP (weight-stationary): preT[64, n] = W1ab_folded^T @ xab
    (fp32r, 512-wide moving tiles), relu+c_vec-bias on Scalar engine,
    PE-transpose back to token-partition layout [128 tok, 64].
    W1ab/b1/c host-permuted (W2>=0 first) and scaled by |W2| so the W2
    stage is reduce(pos) - reduce(neg). gate = sigmoid(logit(nz)+s+b2).
  - payload[128, Sg, MW]: rank 0 initializes via onehot(cm)*gm, ranks
    >=1 accumulate over their slot-prefix; one dma_scatter_add per
    block into the pre-zeroed output (CCE add); pads target a per-block
    pad segment with zero payload.
"""

import sys

import numpy as np

for _p in ("/opt/trn_rl_repo",):
    if _p not in sys.path:
        sys.path.insert(0, _p)

N = 10000
D = 64
NCORES = 8
RPC = N // NCORES  # rows per core
BLK = 64  # rows per block
MW = 128  # merge width (scatter elem size, f32; bytes must be %256)
NSEGW = -(-N // MW)  # real MW-wide segments per row
SEGW = NSEGW + 1  # +1 pad segment (all-zero scatter target)
PITCH = SEGW * MW  # row pitch in the output slab


def _blocks():
    out = []
    r = 0
    while r < RPC:
        h = min(BLK, RPC - r)
        out.append((r, h))
        r += h
    return out


def _prep_host(row, col, noise, embed, adj):
    """Route contributions to (core, block, segment-group, rank)."""
    row = np.asarray(row).astype(np.int64).ravel()
    col = np.asarray(col).astype(np.int64).ravel()
    noise = np.asarray(noise).astype(np.float32).ravel()

    dr = np.concatenate([row, col])  # dest row
    dc = np.concatenate([col, row])  # dest col
    ea = np.concatenate([row, row])  # MLP first input index (edge row)
    eb = np.concatenate([col, col])  # MLP second input index (edge col)
    en = np.concatenate([noise, noise])
    av_all = np.asarray(adj)[dr, dc].astype(np.float32)
    core = dr // RPC

    blocks = _blocks()
    nblk = len(blocks)
    pad_si = NSEGW  # row 0's pad segment; never holds real data

    # Pass 1: per (core, block) group contributions into MW-segments,
    # rank within segment, slot = position of segment in count-desc order.
    info = [[None] * nblk for _ in range(NCORES)]
    for k in range(NCORES):
        m = core == k
        rl = dr[m] - k * RPC
        dcc, a, b, nz, av = dc[m], ea[m], eb[m], en[m], av_all[m]
        blk_id = rl // BLK
        for bi, (r0, h) in enumerate(blocks):
            sel = blk_id == bi
            rls = rl[sel] - r0
            dcs = dcc[sel]
            gsi = rls * SEGW + dcs // MW
            o = np.argsort(gsi, kind="stable")
            gsi_s = gsi[o]
            uq, inv, cnt = np.unique(
                gsi_s, return_inverse=True, return_counts=True
            )
            starts = np.zeros(len(uq) + 1, np.int64)
            np.cumsum(cnt, out=starts[1:])
            rank = np.arange(len(gsi_s)) - starts[inv]
            gord = np.argsort(-cnt, kind="stable")  # groups by count desc
            slot_of_group = np.empty(len(uq), np.int64)
            slot_of_group[gord] = np.arange(len(uq))
            slot = slot_of_group[inv]
            cnt_sorted = cnt[gord]
            maxrank = int(cnt_sorted[0]) if len(cnt_sorted) else 0
            n_j = [int((cnt_sorted > j).sum()) for j in range(maxrank)]
            info[k][bi] = dict(
                a=a[sel][o], b=b[sel][o], nz=nz[sel][o],
                cm=(dcs[o] % MW).astype(np.float32),
                av=av[sel][o], rank=rank, slot=slot, n_j=n_j,
                si_tok=uq[gord], G=len(uq),
            )

    # Pass 2: SPMD-static sizes per block
    chunks = []
    o1 = o16 = o128 = 0
    for bi, (r0, h) in enumerate(blocks):
        Tg = max(info[k][bi]["G"] for k in range(NCORES))
        Tg = max(-(-Tg // 128) * 128, 128)
        Sg0 = Tg // 128
        maxrank = max(len(info[k][bi]["n_j"]) for k in range(NCORES))
        rank_cols = []
        off = Sg0
        for j in range(1, maxrank):
            nj = max(
                (info[k][bi]["n_j"][j] if j < len(info[k][bi]["n_j"]) else 0)
                for k in range(NCORES)
            )
            ncols = -(-nj // 128)
            if ncols <= 0:
                continue
            rank_cols.append((j, off, ncols))
            off += ncols
        S = off
        t = S * 128
        chunks.append(dict(
            bi=bi, r0b=bi * BLK, S=S, Sg0=Sg0, Tg=Tg,
            rank_cols=rank_cols, t=t, o1=o1, o16=o16, o128=o128,
        ))
        o1 += t
        o16 += Tg // 16
        o128 += S
    total1, total16, total128 = o1, o16, o128

    embed = np.asarray(embed, dtype=np.float32)
    embT = np.ascontiguousarray(embed.T)  # [D, N]

    per_core = []
    for k in range(NCORES):
        xab = np.zeros((2 * D, total1), np.float32)
        si16 = np.full((128, total16), pad_si, np.int16)
        nzf = np.full((128, total128), 0.5, np.float32)
        cmf = np.zeros((128, total128), np.float32)
        avf = np.zeros((128, total128), np.float32)
        for ch in chunks:
            nfo = info[k][ch["bi"]]
            t, o1, o16, o128 = ch["t"], ch["o1"], ch["o16"], ch["o128"]
            # stream: rank-major, slot position within rank
            a = np.zeros(t, np.int64)
            b = np.zeros(t, np.int64)
            nz = np.full(t, 0.5, np.float32)
            cm = np.zeros(t, np.float32)
            av = np.zeros(t, np.float32)
            col_off = {0: 0}
            for j, off, ncols in ch["rank_cols"]:
                col_off[j] = off
            for j in range(len(nfo["n_j"])):
                if j not in col_off:
                    continue
                sel = nfo["rank"] == j
                pos = col_off[j] * 128 + nfo["slot"][sel]
                a[pos] = nfo["a"][sel]
                b[pos] = nfo["b"][sel]
                nz[pos] = nfo["nz"][sel]
                cm[pos] = nfo["cm"][sel]
                av[pos] = nfo["av"][sel]
            xab[:D, o1 : o1 + t] = embT[:, a]
            xab[D:, o1 : o1 + t] = embT[:, b]
            si = np.full(ch["Tg"], pad_si, np.int64)
            si[: nfo["G"]] = nfo["si_tok"]
            si16[:, o16 : o16 + ch["Tg"] // 16] = np.tile(
                np.ascontiguousarray(si.reshape(-1, 16).T), (8, 1)
            ).astype(np.int16)
            S = ch["S"]
            nzf[:, o128 : o128 + S] = np.ascontiguousarray(nz.reshape(-1, 128).T)
            cmf[:, o128 : o128 + S] = np.ascontiguousarray(cm.reshape(-1, 128).T)
            avf[:, o128 : o128 + S] = np.ascontiguousarray(av.reshape(-1, 128).T)
        per_core.append(dict(xab=xab, si16=si16, nz=nzf, cm=cmf, av=avf))
    return per_core, chunks, total1, total16, total128


def _build_program(chunks, total1, total16, total128, b2f, pos_cnt):
    import concourse.bacc as bacc
    import concourse.mybir as mybir
    import concourse.tile as tile
    from concourse.masks import make_identity

    f32 = mybir.dt.float32
    bf16 = mybir.dt.bfloat16
    f32r = mybir.dt.float32r
    i16 = mybir.dt.int16
    add = mybir.AluOpType.add
    mult = mybir.AluOpType.mult
    subtract = mybir.AluOpType.subtract
    is_equal = mybir.AluOpType.is_equal
    AF = mybir.ActivationFunctionType

    nc = bacc.Bacc(num_swdge_queues=4)

    blocks = _blocks()
    out_rows = BLK * len(blocks)

    xabp = nc.declare_dram_parameter("xab", [2 * D, total1], f32r, isOutput=False)
    sip = nc.declare_dram_parameter("si16", [128, total16], i16, isOutput=False)
    nzp = nc.declare_dram_parameter("nz", [128, total128], f32, isOutput=False)
    cmp_ = nc.declare_dram_parameter("cm", [128, total128], f32, isOutput=False)
    avp = nc.declare_dram_parameter("av", [128, total128], f32, isOutput=False)
    w1p = nc.declare_dram_parameter("w1abf", [2 * D, D], f32r, isOutput=False)
    w1cp = nc.declare_dram_parameter("w1cf", [D, D], f32, isOutput=False)
    b1p = nc.declare_dram_parameter("b1f", [1, D], f32, isOutput=False)
    e5p = nc.declare_dram_parameter("e5t", [D, 1], f32, isOutput=False)
    iop = nc.declare_dram_parameter("iotaw", [128, MW], f32, isOutput=False)
    outps = [
        nc.declare_dram_parameter(f"out{bi}", [BLK, PITCH], f32, isOutput=True)
        for bi in range(len(blocks))
    ]

    MMT = 512  # moving-dim tile for the W1 matmul

    with tile.TileContext(nc) as tc:
        with (
            tc.tile_pool(name="const", bufs=1) as cp,
            tc.tile_pool(name="xin", bufs=4) as xp,
            tc.tile_pool(name="hts", bufs=4) as hp,
            tc.tile_pool(name="work", bufs=4) as wp,
            tc.tile_pool(name="sidx", bufs=4) as sp,
            tc.tile_pool(name="smal", bufs=4) as mp,
            tc.tile_pool(name="pay", bufs=4) as yp,
            tc.tile_pool(name="psa", bufs=2, space="PSUM") as ppa,
            tc.tile_pool(name="psb", bufs=4, space="PSUM") as ppb,
            tc.tile_pool(name="psc", bufs=1, space="PSUM") as ppc,
        ):
            identity = cp.tile([128, 128], f32)
            make_identity(nc, identity[:])
            w1ab = cp.tile([2 * D, D], f32r)
            nc.sync.dma_start(out=w1ab[:], in_=w1p[:, :])
            w1c = cp.tile([D, D], f32)
            nc.sync.dma_start(out=w1c[:], in_=w1cp[:, :])
            b1t = cp.tile([1, D], f32)
            nc.sync.dma_start(out=b1t[:], in_=b1p[:, :])
            e5 = cp.tile([D, 1], f32)
            nc.sync.dma_start(out=e5[:], in_=e5p[:, :])
            iot = cp.tile([128, MW], f32)
            nc.sync.dma_start(out=iot[:], in_=iop[:, :])

            # c_vec = embed[node_idx] @ W1c_folded + b1_folded -> [64, 1]
            cps = ppc.tile([1, D], f32, tag="cps")
            nc.tensor.matmul(cps[:], lhsT=e5[:], rhs=w1c[:], start=True, stop=True)
            crow = cp.tile([1, D], f32)
            nc.vector.tensor_tensor(out=crow[:], in0=cps[:], in1=b1t[:], op=add)
            cpsT = ppc.tile([D, 1], f32, tag="cpsT")
            nc.tensor.transpose(cpsT[:], crow[:], identity[:1, :1])
            cT = cp.tile([D, 1], f32)
            nc.scalar.copy(out=cT[:], in_=cpsT[:])

            for ch in chunks:
                S, Sg0, Tg, t = ch["S"], ch["Sg0"], ch["Tg"], ch["t"]
                o1, o16, o128, r0b = ch["o1"], ch["o16"], ch["o128"], ch["r0b"]

                nz = mp.tile([128, S], f32, tag="nz")
                nc.sync.dma_start(out=nz[:], in_=nzp[:, o128 : o128 + S])
                cm = mp.tile([128, S], f32, tag="cm")
                nc.sync.dma_start(out=cm[:], in_=cmp_[:, o128 : o128 + S])
                av = mp.tile([128, S], f32, tag="av")
                nc.sync.dma_start(out=av[:], in_=avp[:, o128 : o128 + S])

                # MLP in two sub-chunks to bound SBUF
                h = wp.tile([128, S * D], f32, tag="h")
                sii = sp.tile([128, Tg // 16], i16, tag="sii")
                Sa = -(-S // 2)
                for (c0, cS) in ((0, Sa), (Sa, S - Sa)):
                    if cS <= 0:
                        continue
                    ta = cS * 128
                    xt = xp.tile([2 * D, ta], f32r, tag="xt")
                    nc.sync.dma_start(
                        out=xt[:], in_=xabp[:, o1 + c0 * 128 : o1 + c0 * 128 + ta]
                    )
                    hT = hp.tile([D, ta], f32, tag="hT")
                    for j0 in range(0, ta, MMT):
                        n = min(MMT, ta - j0)
                        psA = ppa.tile([D, MMT], f32, tag="psA")
                        nc.tensor.matmul(
                            psA[:, :n],
                            lhsT=w1ab[:],
                            rhs=xt[:, j0 : j0 + n],
                            start=True,
                            stop=True,
                        )
                        nc.scalar.activation(
                            out=hT[:, j0 : j0 + n], in_=psA[:, :n],
                            func=AF.Relu, bias=cT[:],
                        )
                    for g0 in range(0, cS, 4):
                        gn = min(4, cS - g0)
                        psB = ppb.tile([128, 4 * D], f32, tag="psB")
                        for q in range(gn):
                            nc.tensor.transpose(
                                psB[:, q * D : (q + 1) * D],
                                hT[:, (g0 + q) * 128 : (g0 + q + 1) * 128],
                                identity[:D, :D],
                            )
                        nc.scalar.copy(
                            out=h[:, (c0 + g0) * D : (c0 + g0 + gn) * D],
                            in_=psB[:, : gn * D],
                        )

                nc.sync.dma_start(out=sii[:], in_=sip[:, o16 : o16 + Tg // 16])

                h3 = h[:].rearrange("p (s d) -> p s d", d=D)
                s = wp.tile([128, S], f32, tag="s")
                if pos_cnt == D:
                    nc.vector.tensor_reduce(
                        out=s[:], in_=h3, axis=mybir.AxisListType.X, op=add
                    )
                elif pos_cnt == 0:
                    nc.vector.tensor_reduce(
                        out=s[:], in_=h3, axis=mybir.AxisListType.X, op=add,
                        negate=True,
                    )
                else:
                    nc.vector.tensor_reduce(
                        out=s[:], in_=h3[:, :, :pos_cnt],
                        axis=mybir.AxisListType.X, op=add,
                    )
                    sn = wp.tile([128, S], f32, tag="sn")
                    nc.vector.tensor_reduce(
                        out=sn[:], in_=h3[:, :, pos_cnt:],
                        axis=mybir.AxisListType.X, op=add,
                    )
                    nc.vector.tensor_tensor(
                        out=s[:], in0=s[:], in1=sn[:], op=subtract
                    )

                # gate = sigmoid(ln(nz) - ln(1-nz) + s + b2); gm = 0.5*g*av
                om = wp.tile([128, S], f32, tag="om")
                nc.vector.tensor_scalar(
                    out=om[:], in0=nz[:], scalar1=-1.0, scalar2=1.0,
                    op0=mult, op1=add,
                )
                ln1 = wp.tile([128, S], f32, tag="ln1")
                nc.scalar.activation(out=ln1[:], in_=nz[:], func=AF.Ln)
                ln2 = wp.tile([128, S], f32, tag="ln2")
                nc.scalar.activation(out=ln2[:], in_=om[:], func=AF.Ln)
                z = wp.tile([128, S], f32, tag="z")
                nc.vector.scalar_tensor_tensor(
                    out=z[:], in0=ln1[:], scalar=b2f, in1=ln2[:],
                    op0=add, op1=subtract,
                )
                nc.vector.tensor_tensor(out=z[:], in0=z[:], in1=s[:], op=add)
                g_ = wp.tile([128, S], f32, tag="g")
                nc.scalar.activation(out=g_[:], in_=z[:], func=AF.Sigmoid)
                gm = wp.tile([128, S], f32, tag="gm")
                nc.vector.scalar_tensor_tensor(
                    out=gm[:], in0=g_[:], scalar=0.5, in1=av[:],
                    op0=mult, op1=mult,
                )

                # payload: one full-width onehot*gm over all rank cols,
                # then in-place prefix adds fold rank regions into rank 0
                pay = yp.tile([128, S * MW], f32, tag="pay")
                payall = pay[:].rearrange("p (s w) -> p s w", w=MW)
                pay3 = payall[:, :Sg0, :]
                io_b = iot[:].rearrange("p (o w) -> p o w", o=1)
                nc.vector.tensor_tensor(
                    out=payall,
                    in0=io_b.to_broadcast([128, S, MW]),
                    in1=cm[:].rearrange("p (s o) -> p s o", o=1)
                    .to_broadcast([128, S, MW]),
                    op=is_equal,
                )
                nc.vector.tensor_tensor(
                    out=payall,
                    in0=payall,
                    in1=gm[:].rearrange("p (s o) -> p s o", o=1)
                    .to_broadcast([128, S, MW]),
                    op=mult,
                )
                for j, off, ncols in ch["rank_cols"]:
                    nc.vector.tensor_tensor(
                        out=payall[:, :ncols, :],
                        in0=payall[:, :ncols, :],
                        in1=payall[:, off : off + ncols, :],
                        op=add,
                    )

                out_view = outps[ch["bi"]][:, :].rearrange(
                    "p (s w) -> (p s) w", w=MW
                )
                nc.gpsimd.dma_scatter_add(
                    out_ap=out_view,
                    in_ap=pay3,
                    idxs_ap=sii[:],
                    num_idxs=Tg,
                    num_idxs_reg=Tg,
                    elem_size=MW,
                    queue_num=ch["bi"] % 4,
                )

    nc.compile()
    return nc


def kernel(embed, row, col, adj, noise, W1, b1, W2, b2, node_idx):
    from concourse.bass_utils import run_bass_kernel_spmd

    embed = np.ascontiguousarray(np.asarray(embed), dtype=np.float32)
    adj = np.ascontiguousarray(np.asarray(adj), dtype=np.float32)
    W1 = np.ascontiguousarray(np.asarray(W1), dtype=np.float32)
    b1 = np.ascontiguousarray(np.asarray(b1), dtype=np.float32).ravel()
    W2 = np.ascontiguousarray(np.asarray(W2), dtype=np.float32)
    b2f = float(np.asarray(b2, dtype=np.float32).ravel()[0])
    nidx = int(np.asarray(node_idx))

    # permute hidden units (W2 >= 0 first) and fold |W2| into W1/b1 so
    # the W2 stage becomes reduce(pos) - reduce(neg) after relu
    w2v = W2.reshape(-1).astype(np.float32)
    order = np.argsort(w2v < 0, kind="stable")
    pos_cnt = int((w2v >= 0).sum())
    w2a = np.abs(w2v[order]).reshape(1, D)
    W1f = W1[:, order] * w2a  # [3D, D]
    b1f = (b1[order].reshape(1, D) * w2a).astype(np.float32)
    w1abf = np.ascontiguousarray(W1f[: 2 * D])
    w1cf = np.ascontiguousarray(W1f[2 * D :])
    e5t = np.ascontiguousarray(embed[nidx].reshape(D, 1))
    iotaw = np.ascontiguousarray(
        np.tile(np.arange(MW, dtype=np.float32).reshape(1, MW), (128, 1))
    )

    per_core, chunks, total1, total16, total128 = _prep_host(
        row, col, noise, embed, adj
    )
    nc = _build_program(chunks, total1, total16, total128, b2f, pos_cnt)

    blocks = _blocks()
    in_maps = []
    for k in range(NCORES):
        m = dict(per_core[k])
        m.update(w1abf=w1abf, w1cf=w1cf, b1f=b1f, e5t=e5t, iotaw=iotaw)
        in_maps.append(m)

    res = run_bass_kernel_spmd(nc, in_maps, list(range(NCORES)))
    kernel.last_exec_time_ns = res.exec_time_ns
    pieces = []
    for k in range(NCORES):
        for bi, (r0, h) in enumerate(blocks):
            pieces.append(res.results[k][f"out{bi}"][:h, :N])
    out = np.concatenate(pieces, axis=0)
    return out


kernel.last_exec_time_ns = None
